# revision 18
# baseline (speedup 1.0000x reference)
"""Trainium2 Bass kernel: 34-channel per-channel GRU (input_size=1) over ragged
sequences + concat features -> linear proj -> BatchNorm(train fwd) -> ReLU ->
linear -> sigmoid.

Strategy (v2):
  - Channel-parallel across the 8 NeuronCores: C=34 padded to 40, 5 channels
    per core, full batch B=256 everywhere. Params replicated per-slice.
  - Batch sorted by lens (descending) on the host; at step t only the active
    prefix n_t = #{lens > t} of columns is computed (exact ragged freeze).
  - x rows are staged through two SBUF ring tiles of R=16 step-slots each
    (h in partitions 0:64, x_t in partition 64, ones in partition 65); one DMA
    per 16 steps refills a ring two windows ahead, so no DMA is ever on the
    per-step critical path.
  - The hidden state h lives in the same ring tiles and advances slot-to-slot;
    when columns drop out (sample length reached) the final h for those
    columns is written to a fixed `hfinal` tile instead of the next slot.
  - Channels are split into 3 groups (2,2,1) per core; each group is an
    independent software pipeline, which shortens the per-step dependency
    chain (mm -> sigmoid -> r*nh -> id-matmul accumulate -> tanh -> d,p,h).
  - Per channel, one [66,128] matmul produces [z|r] gate preacts and one
    produces [nx|nh]; contraction rows are [h(64); x_t(1); ones(1)] so the
    input contribution and biases ride in the same matmul.
  - Final features -> partial projection per core -> AllReduce -> BatchNorm
    (batch stats; proj bias cancels in BN) -> ReLU -> pred matvec -> sigmoid.
"""

import sys

sys.path.insert(0, "/opt/trn_rl_repo")

import numpy as np
import ml_dtypes

import concourse.bass as bass
from concourse import bacc, mybir
from concourse.tile import TileContext
from concourse.bass_utils import run_bass_kernel_spmd

B, T, C, H = 256, 512, 34, 64
EPS = 1e-5
NCORES = 8
CPAD = 40  # padded channels
CH_LOC = CPAD // NCORES  # 5
GROUPS = [(0, 3), (3, 2)]  # (start_channel, n_channels) per pipeline
R = 16  # ring slots (steps) per window

BF16 = mybir.dt.bfloat16
F32 = mybir.dt.float32
bfnp = ml_dtypes.bfloat16
AF = mybir.ActivationFunctionType
OP = mybir.AluOpType

W_COLS = R * CH_LOC * B  # ring tile columns


def _v3(ap2, nch, n, p0=None, p1=None, c0=0):
    """[P, CH*B] AP -> [p0:p1, c0:c0+nch, 0:n] 3D view with B-col channels."""
    a = ap2.rearrange("p (c b) -> p c b", b=B)
    if p0 is None:
        return a[:, c0 : c0 + nch, 0:n]
    return a[p0:p1, c0 : c0 + nch, 0:n]


def _vc3(ap2, nch, n, p0=None, p1=None, lo=0, hi=None):
    """compact work tile [P, W] with channel stride n -> [p, nch, lo:hi]."""
    hi = n if hi is None else hi
    a = ap2[:, 0 : nch * n].rearrange("p (c b) -> p c b", c=nch)
    if p0 is None:
        return a[:, :, lo:hi]
    return a[p0:p1, :, lo:hi]


def build_program(t_eff, nsched):
    nwin = (t_eff + R - 1) // R
    nc = bacc.Bacc(
        "TRN2", target_bir_lowering=False, debug=False, num_devices=NCORES
    )
    xw_d = nc.dram_tensor("xw", [nwin * 2, W_COLS], BF16, kind="ExternalInput").ap()
    wrz_d = nc.dram_tensor("wrz", [66, CH_LOC * 128], BF16, kind="ExternalInput").ap()
    wnhx_d = nc.dram_tensor("wnhx", [66, CH_LOC * 128], BF16, kind="ExternalInput").ap()
    wp_d = nc.dram_tensor("wp", [64, CH_LOC * 64], BF16, kind="ExternalInput").ap()
    wpred_d = nc.dram_tensor("wpred", [64, 1], BF16, kind="ExternalInput").ap()
    ident_d = nc.dram_tensor("ident", [64, 64], BF16, kind="ExternalInput").ap()
    scale_d = nc.dram_tensor("scalecol", [128, 1], F32, kind="ExternalInput").ap()
    gamma_d = nc.dram_tensor("gammacol", [64, 1], F32, kind="ExternalInput").ap()
    beta_d = nc.dram_tensor("betacol", [64, 1], F32, kind="ExternalInput").ap()
    bpred_d = nc.dram_tensor("bpredcol", [1, 1], F32, kind="ExternalInput").ap()
    out_d = nc.dram_tensor("out", [1, B], F32, kind="ExternalOutput").ap()
    cc_in = nc.dram_tensor("cc_in", [64, B], F32).ap()
    cc_out = nc.dram_tensor("cc_out", [64, B], F32, addr_space="Shared").ap()

    with TileContext(nc) as tc:
        with (
            tc.tile_pool(name="const", bufs=1) as cp,
            tc.tile_pool(name="work", bufs=2) as wk,
            tc.tile_pool(name="psum", bufs=1, space="PSUM") as pp,
        ):
            wrz = cp.tile([66, CH_LOC * 128], BF16)
            nc.gpsimd.dma_start(wrz[:], wrz_d[:])
            wnhx = cp.tile([66, CH_LOC * 128], BF16)
            nc.gpsimd.dma_start(wnhx[:], wnhx_d[:])
            ident = cp.tile([64, 64], BF16)
            nc.gpsimd.dma_start(ident[:], ident_d[:])
            scol = cp.tile([128, 1], F32)
            nc.gpsimd.dma_start(scol[:], scale_d[:])

            ring_a = cp.tile([66, W_COLS], BF16)
            ring_b = cp.tile([66, W_COLS], BF16)
            rings = [ring_a, ring_b]
            hfinal = cp.tile([64, CH_LOC * B], BF16)

            # h0 = 0 in ring0 slot 0; x/ones for windows 0,1 via DMA
            nc.vector.memset(rings[0][0:64, 0 : CH_LOC * B], 0.0)
            nc.gpsimd.dma_start(rings[0][64:66, :], xw_d[0:2, :])
            if nwin > 1:
                nc.gpsimd.dma_start(rings[1][64:66, :], xw_d[2:4, :])

            def group_step(t, c0, ng, tiles):
                """Emit one GRU step for channels [c0, c0+ng)."""
                n = int(nsched[t])
                s = t % R
                w = t // R
                ring = rings[w % 2]
                s2 = (t + 1) % R
                ring2 = rings[((t + 1) // R) % 2]
                cb0 = (s * CH_LOC + c0) * B
                nb2 = (s2 * CH_LOC + c0) * B
                hcur = ring[:, cb0 : cb0 + ng * B]
                hnxt = ring2[:, nb2 : nb2 + ng * B]
                gi = c0
                rz0, t10, nh0, d0, p0 = tiles
                rz = rz0[:, c0 * B : (c0 + ng) * B]
                t1 = t10[:, c0 * B : (c0 + ng) * B]
                nh = nh0[:, c0 * B : (c0 + ng) * B]
                d = d0[:, c0 * B : (c0 + ng) * B]
                p = p0[:, c0 * B : (c0 + ng) * B]
                # psum padded to full banks (512 f32 cols) per group
                nbk = 2 * ((ng + 1) // 2)
                arz = pp.tile([128, nbk * B], F32, tag=f"arz{gi}")
                anhx = pp.tile([128, nbk * B], F32, tag=f"anhx{gi}")
                for j in range(ng):
                    c = c0 + j
                    nc.tensor.matmul(
                        arz[:, j * B : j * B + n],
                        wrz[:, c * 128 : (c + 1) * 128],
                        ring[0:66, cb0 + j * B : cb0 + j * B + n],
                        start=True,
                        stop=True,
                    )
                for j in range(ng):
                    c = c0 + j
                    # start=True clears has_written for the WHOLE psum bank;
                    # only the first channel of each bank may clear it.
                    nc.tensor.matmul(
                        anhx[:, j * B : j * B + n],
                        wnhx[:, c * 128 : (c + 1) * 128],
                        ring[0:66, cb0 + j * B : cb0 + j * B + n],
                        start=(j % 2 == 0),
                        stop=False,
                        skip_group_check=True,
                    )
                # sigmoid with per-partition scale [-1]*64 ++ [+1]*64:
                # rows 0:64 = zbar = sig(-Az), rows 64:128 = r = sig(Ar)
                nc.scalar.activation(
                    _v3(rz, ng, n),
                    _v3(arz, ng, n),
                    AF.Sigmoid,
                    scale=scol[:, 0:1],
                )
                # t1 = r * nh   (nh lives on psum partitions 64:128)
                nc.vector.tensor_mul(
                    _v3(t1, ng, n, 0, 64),
                    _v3(rz, ng, n, 64, 128),
                    _v3(anhx, ng, n, 64, 128),
                )
                # nx += I @ t1: one identity-matmul per psum bank pair
                # (psum accumulate, B-strided out; must not cross banks)
                for j0 in range(0, ng, 2):
                    pn = min(2, ng - j0)
                    nc.tensor.matmul(
                        _v3(anhx, pn, n, 0, 64, c0=j0),
                        ident[0:64, :],
                        _v3(t1, pn, n, 0, 64, c0=j0),
                        start=False,
                        stop=True,
                        skip_group_check=True,
                    )
                nc.scalar.activation(
                    _v3(nh, ng, n), _v3(anhx, ng, n, 0, 64), AF.Tanh
                )
                nc.vector.tensor_sub(
                    _v3(d, ng, n), _v3(nh, ng, n), _v3(hcur, ng, n, 0, 64)
                )
                nc.vector.tensor_mul(
                    _v3(p, ng, n), _v3(rz, ng, n, 0, 64), _v3(d, ng, n)
                )
                nc.vector.tensor_add(
                    _v3(hnxt, ng, n, 0, 64),
                    _v3(hcur, ng, n, 0, 64),
                    _v3(p, ng, n),
                )

            # groups run phase-skewed (g1 at t, g2 at t-1, g3 at t-2) so a
            # slow group never head-of-line-blocks another group's ready
            # instructions in the in-order engine queues.
            for t in range(t_eff):
                n = int(nsched[t])
                n2 = int(nsched[t + 1]) if t + 1 < t_eff else 0
                rz0 = wk.tile([128, CH_LOC * B], BF16, tag="rz")
                t10 = wk.tile([64, CH_LOC * B], BF16, tag="t1")
                nh0 = wk.tile([64, CH_LOC * B], BF16, tag="nh")
                d0 = wk.tile([64, CH_LOC * B], BF16, tag="d")
                p0 = wk.tile([64, CH_LOC * B], BF16, tag="p")
                tiles = (rz0, t10, nh0, d0, p0)
                for (c0, ng) in GROUPS:
                    group_step(t, c0, ng, tiles)
                if n2 < n:
                    # retire frozen columns n2:n of all channels to hfinal
                    s2 = (t + 1) % R
                    ring2 = rings[((t + 1) // R) % 2]
                    a_hf = hfinal.rearrange("p (c b) -> p c b", c=CH_LOC)[
                        :, :, n2:n
                    ]
                    a_h2 = ring2[
                        :, s2 * CH_LOC * B : (s2 + 1) * CH_LOC * B
                    ].rearrange("p (c b) -> p c b", c=CH_LOC)[0:64, :, n2:n]
                    nc.gpsimd.tensor_copy(a_hf, a_h2)
                # prefetch x window w+2 into the ring this window just freed
                if t % R == R - 1:
                    w = t // R
                    if w + 2 < nwin:
                        nc.gpsimd.dma_start(
                            rings[w % 2][64:66, :],
                            xw_d[2 * (w + 2) : 2 * (w + 2) + 2, :],
                        )

            # ---- tail: proj partial -> allreduce -> BN -> relu -> pred ----
            wp = cp.tile([64, CH_LOC * 64], BF16)
            nc.gpsimd.dma_start(wp[:], wp_d[:])
            pj = pp.tile([64, B], F32, tag="pproj")
            for c in range(CH_LOC):
                nc.tensor.matmul(
                    pj[:, :],
                    wp[:, c * 64 : (c + 1) * 64],
                    hfinal[0:64, c * B : (c + 1) * B],
                    start=(c == 0),
                    stop=(c == CH_LOC - 1),
                )
            pjs = cp.tile([64, B], F32)
            nc.scalar.copy(pjs[:], pj[:])
            nc.gpsimd.dma_start(cc_in[:], pjs[:])
            nc.gpsimd.collective_compute(
                "AllReduce",
                OP.add,
                replica_groups=[list(range(NCORES))],
                ins=[cc_in[:]],
                outs=[cc_out[:]],
            )
            prj = cp.tile([64, B], F32)
            nc.gpsimd.dma_start(prj[:], cc_out[:])

            musum = cp.tile([64, 1], F32)
            nc.vector.tensor_reduce(musum[:], prj[:], mybir.AxisListType.X, OP.add)
            mu = cp.tile([64, 1], F32)
            nc.scalar.mul(mu[:], musum[:], 1.0 / B)
            cen = cp.tile([64, B], F32)
            nc.vector.tensor_scalar_sub(cen[:], prj[:], mu[:, 0:1])
            sq = cp.tile([64, B], F32)
            nc.vector.tensor_mul(sq[:], cen[:], cen[:])
            vsum = cp.tile([64, 1], F32)
            nc.vector.tensor_reduce(vsum[:], sq[:], mybir.AxisListType.X, OP.add)
            v = cp.tile([64, 1], F32)
            nc.scalar.mul(v[:], vsum[:], 1.0 / B)
            veps = cp.tile([64, 1], F32)
            nc.vector.tensor_scalar_add(veps[:], v[:], EPS)
            std = cp.tile([64, 1], F32)
            nc.scalar.activation(std[:], veps[:], AF.Sqrt)
            rstd = cp.tile([64, 1], F32)
            nc.vector.reciprocal(rstd[:], std[:])
            gam = cp.tile([64, 1], F32)
            nc.gpsimd.dma_start(gam[:], gamma_d[:])
            bet = cp.tile([64, 1], F32)
            nc.gpsimd.dma_start(bet[:], beta_d[:])
            sc2 = cp.tile([64, 1], F32)
            nc.vector.tensor_mul(sc2[:], rstd[:], gam[:])
            y = cp.tile([64, B], BF16)
            nc.vector.tensor_scalar(
                y[:], cen[:], sc2[:, 0:1], bet[:, 0:1], OP.mult, OP.add
            )
            yr = cp.tile([64, B], BF16)
            nc.vector.tensor_scalar_max(yr[:], y[:], 0.0)
            wpred = cp.tile([64, 1], BF16)
            nc.gpsimd.dma_start(wpred[:], wpred_d[:])
            pps = pp.tile([1, B], F32, tag="pred")
            nc.tensor.matmul(pps[:], wpred[:, 0:1], yr[:, :], start=True, stop=True)
            bp = cp.tile([1, 1], F32)
            nc.gpsimd.dma_start(bp[:], bpred_d[:])
            osb = cp.tile([1, B], F32)
            nc.scalar.activation(osb[:], pps[:], AF.Sigmoid, bias=bp[0:1, 0:1])
            nc.gpsimd.dma_start(out_d[:], osb[:])

    nc.compile()
    return nc


def prepare_inputs(x, Wih, Whh, b_ih, b_hh, Wp, bp, gamma, beta, Wpred, bpred, lens):
    """Host-side: sort batch by lens desc, pack per-core tensors."""
    x = np.asarray(x)
    lens = np.asarray(lens)
    perm = np.argsort(-lens, kind="stable")
    lens_s = lens[perm]
    x_s = x[perm]  # [B, T, C]

    nsched = np.array([(lens_s > t).sum() for t in range(T)], dtype=np.int64)
    t_eff = int((nsched > 0).sum())
    nsched = nsched[:t_eff]
    nwin = (t_eff + R - 1) // R

    # padded params
    WihP = np.zeros((CPAD, 3 * H), np.float32)
    WihP[:C] = np.asarray(Wih)
    WhhP = np.zeros((CPAD, 3 * H, H), np.float32)
    WhhP[:C] = np.asarray(Whh)
    bihP = np.zeros((CPAD, 3 * H), np.float32)
    bihP[:C] = np.asarray(b_ih)
    bhhP = np.zeros((CPAD, 3 * H), np.float32)
    bhhP[:C] = np.asarray(b_hh)
    WpP = np.zeros((H, CPAD * H), np.float32)
    WpP[:, : C * H] = np.asarray(Wp)

    scale_col = np.concatenate(
        [-np.ones((64, 1), np.float32), np.ones((64, 1), np.float32)]
    )
    ident = np.eye(64, dtype=bfnp)

    in_maps = []
    for k in range(NCORES):
        gs = list(range(k * CH_LOC, (k + 1) * CH_LOC))
        xrow = np.zeros((nwin * R, CH_LOC, B), np.float32)
        wrz = np.zeros((66, CH_LOC * 128), np.float32)
        wnhx = np.zeros((66, CH_LOC * 128), np.float32)
        wp_t = np.zeros((64, CH_LOC * 64), np.float32)
        for c, g in enumerate(gs):
            if g < C:
                xrow[:t_eff, c, :] = x_s[:, :t_eff, g].T
            o = c * 128
            # z block (cols 0:64), r block (cols 64:128)
            wrz[0:64, o : o + 64] = WhhP[g, H : 2 * H, :].T
            wrz[64, o : o + 64] = WihP[g, H : 2 * H]
            wrz[65, o : o + 64] = bihP[g, H : 2 * H] + bhhP[g, H : 2 * H]
            wrz[0:64, o + 64 : o + 128] = WhhP[g, 0:H, :].T
            wrz[64, o + 64 : o + 128] = WihP[g, 0:H]
            wrz[65, o + 64 : o + 128] = bihP[g, 0:H] + bhhP[g, 0:H]
            # nx block (cols 0:64), nh block (cols 64:128)
            wnhx[64, o : o + 64] = WihP[g, 2 * H : 3 * H]
            wnhx[65, o : o + 64] = bihP[g, 2 * H : 3 * H]
            wnhx[0:64, o + 64 : o + 128] = WhhP[g, 2 * H : 3 * H, :].T
            wnhx[65, o + 64 : o + 128] = bhhP[g, 2 * H : 3 * H]
            wp_t[:, c * 64 : (c + 1) * 64] = WpP[:, g * H : (g + 1) * H].T
        # window-packed x: [nwin, 2, R*CH_LOC*B] rows (x, ones)
        xw = np.empty((nwin, 2, W_COLS), np.float32)
        xw[:, 0, :] = xrow.reshape(nwin, R * CH_LOC * B)
        xw[:, 1, :] = 1.0
        in_maps.append(
            {
                "xw": xw.reshape(nwin * 2, W_COLS).astype(bfnp),
                "wrz": wrz.astype(bfnp),
                "wnhx": wnhx.astype(bfnp),
                "wp": wp_t.astype(bfnp),
                "wpred": np.asarray(Wpred, np.float32).reshape(1, 64).T.astype(bfnp),
                "ident": ident,
                "scalecol": scale_col,
                "gammacol": np.asarray(gamma, np.float32).reshape(64, 1),
                "betacol": np.asarray(beta, np.float32).reshape(64, 1),
                "bpredcol": np.asarray(bpred, np.float32).reshape(1, 1),
            }
        )
    return in_maps, perm, t_eff, nsched


_CACHE = {}


def run(inputs, trace=False):
    in_maps, perm, t_eff, nsched = prepare_inputs(**inputs)
    key = (t_eff, tuple(int(v) for v in nsched))
    if key not in _CACHE:
        _CACHE[key] = build_program(t_eff, nsched)
    nc = _CACHE[key]
    res = run_bass_kernel_spmd(
        nc, in_maps, list(range(NCORES)), trace=trace
    )
    out_sorted = np.asarray(res.results[0]["out"][0], np.float32)
    out = np.empty(B, np.float32)
    out[perm] = out_sorted
    return out, res


def kernel(**inputs):
    out, _ = run(inputs, trace=False)
    return out


# revision 20
# speedup vs baseline: 1.0897x; 1.0897x over previous
"""Trainium2 Bass kernel: 34-channel per-channel GRU (input_size=1) over ragged
sequences + concat features -> linear proj -> BatchNorm(train fwd) -> ReLU ->
linear -> sigmoid.

Strategy (v2):
  - Channel-parallel across the 8 NeuronCores: C=34 padded to 40, 5 channels
    per core, full batch B=256 everywhere. Params replicated per-slice.
  - Batch sorted by lens (descending) on the host; at step t only the active
    prefix n_t = #{lens > t} of columns is computed (exact ragged freeze).
  - x rows are staged through two SBUF ring tiles of R=16 step-slots each
    (h in partitions 0:64, x_t in partition 64, ones in partition 65); one DMA
    per 16 steps refills a ring two windows ahead, so no DMA is ever on the
    per-step critical path.
  - The hidden state h lives in the same ring tiles and advances slot-to-slot;
    when columns drop out (sample length reached) the final h for those
    columns is written to a fixed `hfinal` tile instead of the next slot.
  - Channels are split into 3 groups (2,2,1) per core; each group is an
    independent software pipeline, which shortens the per-step dependency
    chain (mm -> sigmoid -> r*nh -> id-matmul accumulate -> tanh -> d,p,h).
  - Per channel, one [66,128] matmul produces [z|r] gate preacts and one
    produces [nx|nh]; contraction rows are [h(64); x_t(1); ones(1)] so the
    input contribution and biases ride in the same matmul.
  - Final features -> partial projection per core -> AllReduce -> BatchNorm
    (batch stats; proj bias cancels in BN) -> ReLU -> pred matvec -> sigmoid.
"""

import sys

sys.path.insert(0, "/opt/trn_rl_repo")

import numpy as np
import ml_dtypes

import concourse.bass as bass
from concourse import bacc, mybir
from concourse.tile import TileContext
from concourse.bass_utils import run_bass_kernel_spmd

B, T, C, H = 256, 512, 34, 64
EPS = 1e-5
NCORES = 8
CPAD = 40  # padded channels
CH_LOC = CPAD // NCORES  # 5
GROUPS = [(0, 1), (1, 1), (2, 1), (3, 1), (4, 1)]  # per pipeline
R = 16  # ring slots (steps) per window

BF16 = mybir.dt.bfloat16
F32 = mybir.dt.float32
bfnp = ml_dtypes.bfloat16
AF = mybir.ActivationFunctionType
OP = mybir.AluOpType

W_COLS = R * CH_LOC * B  # ring tile columns


def _v3(ap2, nch, n, p0=None, p1=None, c0=0):
    """[P, CH*B] AP -> [p0:p1, c0:c0+nch, 0:n] 3D view with B-col channels."""
    a = ap2.rearrange("p (c b) -> p c b", b=B)
    if p0 is None:
        return a[:, c0 : c0 + nch, 0:n]
    return a[p0:p1, c0 : c0 + nch, 0:n]


def _vc3(ap2, nch, n, p0=None, p1=None, lo=0, hi=None):
    """compact work tile [P, W] with channel stride n -> [p, nch, lo:hi]."""
    hi = n if hi is None else hi
    a = ap2[:, 0 : nch * n].rearrange("p (c b) -> p c b", c=nch)
    if p0 is None:
        return a[:, :, lo:hi]
    return a[p0:p1, :, lo:hi]


def build_program(t_eff, nsched):
    nwin = (t_eff + R - 1) // R
    nc = bacc.Bacc(
        "TRN2", target_bir_lowering=False, debug=False, num_devices=NCORES
    )
    xw_d = nc.dram_tensor("xw", [nwin * 2, W_COLS], BF16, kind="ExternalInput").ap()
    wrz_d = nc.dram_tensor("wrz", [66, CH_LOC * 128], BF16, kind="ExternalInput").ap()
    wnhx_d = nc.dram_tensor("wnhx", [66, CH_LOC * 128], BF16, kind="ExternalInput").ap()
    wp_d = nc.dram_tensor("wp", [64, CH_LOC * 64], BF16, kind="ExternalInput").ap()
    wpred_d = nc.dram_tensor("wpred", [64, 1], BF16, kind="ExternalInput").ap()
    ident_d = nc.dram_tensor("ident", [64, 64], BF16, kind="ExternalInput").ap()
    scale_d = nc.dram_tensor("scalecol", [128, 1], F32, kind="ExternalInput").ap()
    gamma_d = nc.dram_tensor("gammacol", [64, 1], F32, kind="ExternalInput").ap()
    beta_d = nc.dram_tensor("betacol", [64, 1], F32, kind="ExternalInput").ap()
    bpred_d = nc.dram_tensor("bpredcol", [1, 1], F32, kind="ExternalInput").ap()
    out_d = nc.dram_tensor("out", [1, B], F32, kind="ExternalOutput").ap()
    cc_in = nc.dram_tensor("cc_in", [64, B], F32).ap()
    cc_out = nc.dram_tensor("cc_out", [64, B], F32, addr_space="Shared").ap()

    with TileContext(nc) as tc:
        with (
            tc.tile_pool(name="const", bufs=1) as cp,
            tc.tile_pool(name="work", bufs=2) as wk,
            tc.tile_pool(name="psum", bufs=1, space="PSUM") as pp,
        ):
            wrz = cp.tile([66, CH_LOC * 128], BF16)
            nc.gpsimd.dma_start(wrz[:], wrz_d[:])
            wnhx = cp.tile([66, CH_LOC * 128], BF16)
            nc.gpsimd.dma_start(wnhx[:], wnhx_d[:])
            ident = cp.tile([64, 64], BF16)
            nc.gpsimd.dma_start(ident[:], ident_d[:])
            scol = cp.tile([128, 1], F32)
            nc.gpsimd.dma_start(scol[:], scale_d[:])

            ring_a = cp.tile([66, W_COLS], BF16)
            ring_b = cp.tile([66, W_COLS], BF16)
            rings = [ring_a, ring_b]
            hfinal = cp.tile([64, CH_LOC * B], BF16)

            # h0 = 0 in ring0 slot 0; x/ones for windows 0,1 via DMA
            nc.vector.memset(rings[0][0:64, 0 : CH_LOC * B], 0.0)
            nc.gpsimd.dma_start(rings[0][64:66, :], xw_d[0:2, :])
            if nwin > 1:
                nc.gpsimd.dma_start(rings[1][64:66, :], xw_d[2:4, :])

            def group_step(t, c0, ng, tiles):
                """Emit one GRU step for channels [c0, c0+ng)."""
                n = int(nsched[t])
                s = t % R
                w = t // R
                ring = rings[w % 2]
                s2 = (t + 1) % R
                ring2 = rings[((t + 1) // R) % 2]
                cb0 = (s * CH_LOC + c0) * B
                nb2 = (s2 * CH_LOC + c0) * B
                hcur = ring[:, cb0 : cb0 + ng * B]
                hnxt = ring2[:, nb2 : nb2 + ng * B]
                gi = c0
                rz0, t10, nh0, d0, p0 = tiles
                rz = rz0[:, c0 * B : (c0 + ng) * B]
                t1 = t10[:, c0 * B : (c0 + ng) * B]
                nh = nh0[:, c0 * B : (c0 + ng) * B]
                d = d0[:, c0 * B : (c0 + ng) * B]
                p = p0[:, c0 * B : (c0 + ng) * B]
                # psum padded to full banks (512 f32 cols) per group;
                # an ng=1 group packs arz|anhx into a single bank (the arz
                # matmul's start=True clears the bank, the nhx matmul then
                # initializes its half via the cleared has_written bits)
                if ng == 1:
                    ab = pp.tile([128, 2 * B], F32, tag=f"ab{gi}")
                    arz = ab[:, 0:B]
                    anhx = ab[:, B : 2 * B]
                else:
                    nbk = 2 * ((ng + 1) // 2)
                    arz = pp.tile([128, nbk * B], F32, tag=f"arz{gi}")
                    anhx = pp.tile([128, nbk * B], F32, tag=f"anhx{gi}")
                for j in range(ng):
                    c = c0 + j
                    nc.tensor.matmul(
                        arz[:, j * B : j * B + n],
                        wrz[:, c * 128 : (c + 1) * 128],
                        ring[0:66, cb0 + j * B : cb0 + j * B + n],
                        start=True,
                        stop=True,
                    )
                for j in range(ng):
                    c = c0 + j
                    # start=True clears has_written for the WHOLE psum bank;
                    # only the first channel of each bank may clear it.
                    nc.tensor.matmul(
                        anhx[:, j * B : j * B + n],
                        wnhx[:, c * 128 : (c + 1) * 128],
                        ring[0:66, cb0 + j * B : cb0 + j * B + n],
                        start=(j % 2 == 0) and ng > 1,
                        stop=False,
                        skip_group_check=True,
                    )
                # sigmoid with per-partition scale [-1]*64 ++ [+1]*64:
                # rows 0:64 = zbar = sig(-Az), rows 64:128 = r = sig(Ar)
                nc.scalar.activation(
                    _v3(rz, ng, n),
                    _v3(arz, ng, n),
                    AF.Sigmoid,
                    scale=scol[:, 0:1],
                )
                # t1 = r * nh   (nh lives on psum partitions 64:128)
                nc.vector.tensor_mul(
                    _v3(t1, ng, n, 0, 64),
                    _v3(rz, ng, n, 64, 128),
                    _v3(anhx, ng, n, 64, 128),
                )
                # nx += I @ t1: one identity-matmul per psum bank pair
                # (psum accumulate, B-strided out; must not cross banks)
                for j0 in range(0, ng, 2):
                    pn = min(2, ng - j0)
                    nc.tensor.matmul(
                        _v3(anhx, pn, n, 0, 64, c0=j0),
                        ident[0:64, :],
                        _v3(t1, pn, n, 0, 64, c0=j0),
                        start=False,
                        stop=True,
                        skip_group_check=True,
                    )
                nc.scalar.activation(
                    _v3(nh, ng, n), _v3(anhx, ng, n, 0, 64), AF.Tanh
                )
                nc.vector.tensor_sub(
                    _v3(d, ng, n), _v3(nh, ng, n), _v3(hcur, ng, n, 0, 64)
                )
                nc.vector.tensor_mul(
                    _v3(p, ng, n), _v3(rz, ng, n, 0, 64), _v3(d, ng, n)
                )
                nc.vector.tensor_add(
                    _v3(hnxt, ng, n, 0, 64),
                    _v3(hcur, ng, n, 0, 64),
                    _v3(p, ng, n),
                )

            # groups run phase-skewed (g1 at t, g2 at t-1, g3 at t-2) so a
            # slow group never head-of-line-blocks another group's ready
            # instructions in the in-order engine queues.
            for t in range(t_eff):
                n = int(nsched[t])
                n2 = int(nsched[t + 1]) if t + 1 < t_eff else 0
                rz0 = wk.tile([128, CH_LOC * B], BF16, tag="rz")
                t10 = wk.tile([64, CH_LOC * B], BF16, tag="t1")
                nh0 = wk.tile([64, CH_LOC * B], BF16, tag="nh")
                d0 = wk.tile([64, CH_LOC * B], BF16, tag="d")
                p0 = wk.tile([64, CH_LOC * B], BF16, tag="p")
                tiles = (rz0, t10, nh0, d0, p0)
                for (c0, ng) in GROUPS:
                    group_step(t, c0, ng, tiles)
                if n2 < n:
                    # retire frozen columns n2:n of all channels to hfinal
                    s2 = (t + 1) % R
                    ring2 = rings[((t + 1) // R) % 2]
                    a_hf = hfinal.rearrange("p (c b) -> p c b", c=CH_LOC)[
                        :, :, n2:n
                    ]
                    a_h2 = ring2[
                        :, s2 * CH_LOC * B : (s2 + 1) * CH_LOC * B
                    ].rearrange("p (c b) -> p c b", c=CH_LOC)[0:64, :, n2:n]
                    nc.gpsimd.tensor_copy(a_hf, a_h2)
                # prefetch x window w+2 into the ring this window just freed
                if t % R == R - 1:
                    w = t // R
                    if w + 2 < nwin:
                        nc.gpsimd.dma_start(
                            rings[w % 2][64:66, :],
                            xw_d[2 * (w + 2) : 2 * (w + 2) + 2, :],
                        )

            # ---- tail: proj partial -> allreduce -> BN -> relu -> pred ----
            wp = cp.tile([64, CH_LOC * 64], BF16)
            nc.gpsimd.dma_start(wp[:], wp_d[:])
            pj = pp.tile([64, B], F32, tag="pproj")
            for c in range(CH_LOC):
                nc.tensor.matmul(
                    pj[:, :],
                    wp[:, c * 64 : (c + 1) * 64],
                    hfinal[0:64, c * B : (c + 1) * B],
                    start=(c == 0),
                    stop=(c == CH_LOC - 1),
                )
            pjs = cp.tile([64, B], F32)
            nc.scalar.copy(pjs[:], pj[:])
            nc.gpsimd.dma_start(cc_in[:], pjs[:])
            nc.gpsimd.collective_compute(
                "AllReduce",
                OP.add,
                replica_groups=[list(range(NCORES))],
                ins=[cc_in[:]],
                outs=[cc_out[:]],
            )
            prj = cp.tile([64, B], F32)
            nc.gpsimd.dma_start(prj[:], cc_out[:])

            musum = cp.tile([64, 1], F32)
            nc.vector.tensor_reduce(musum[:], prj[:], mybir.AxisListType.X, OP.add)
            mu = cp.tile([64, 1], F32)
            nc.scalar.mul(mu[:], musum[:], 1.0 / B)
            cen = cp.tile([64, B], F32)
            nc.vector.tensor_scalar_sub(cen[:], prj[:], mu[:, 0:1])
            sq = cp.tile([64, B], F32)
            nc.vector.tensor_mul(sq[:], cen[:], cen[:])
            vsum = cp.tile([64, 1], F32)
            nc.vector.tensor_reduce(vsum[:], sq[:], mybir.AxisListType.X, OP.add)
            v = cp.tile([64, 1], F32)
            nc.scalar.mul(v[:], vsum[:], 1.0 / B)
            veps = cp.tile([64, 1], F32)
            nc.vector.tensor_scalar_add(veps[:], v[:], EPS)
            std = cp.tile([64, 1], F32)
            nc.scalar.activation(std[:], veps[:], AF.Sqrt)
            rstd = cp.tile([64, 1], F32)
            nc.vector.reciprocal(rstd[:], std[:])
            gam = cp.tile([64, 1], F32)
            nc.gpsimd.dma_start(gam[:], gamma_d[:])
            bet = cp.tile([64, 1], F32)
            nc.gpsimd.dma_start(bet[:], beta_d[:])
            sc2 = cp.tile([64, 1], F32)
            nc.vector.tensor_mul(sc2[:], rstd[:], gam[:])
            y = cp.tile([64, B], BF16)
            nc.vector.tensor_scalar(
                y[:], cen[:], sc2[:, 0:1], bet[:, 0:1], OP.mult, OP.add
            )
            yr = cp.tile([64, B], BF16)
            nc.vector.tensor_scalar_max(yr[:], y[:], 0.0)
            wpred = cp.tile([64, 1], BF16)
            nc.gpsimd.dma_start(wpred[:], wpred_d[:])
            pps = pp.tile([1, B], F32, tag="pred")
            nc.tensor.matmul(pps[:], wpred[:, 0:1], yr[:, :], start=True, stop=True)
            bp = cp.tile([1, 1], F32)
            nc.gpsimd.dma_start(bp[:], bpred_d[:])
            osb = cp.tile([1, B], F32)
            nc.scalar.activation(osb[:], pps[:], AF.Sigmoid, bias=bp[0:1, 0:1])
            nc.gpsimd.dma_start(out_d[:], osb[:])

    nc.compile()
    return nc


def prepare_inputs(x, Wih, Whh, b_ih, b_hh, Wp, bp, gamma, beta, Wpred, bpred, lens):
    """Host-side: sort batch by lens desc, pack per-core tensors."""
    x = np.asarray(x)
    lens = np.asarray(lens)
    perm = np.argsort(-lens, kind="stable")
    lens_s = lens[perm]
    x_s = x[perm]  # [B, T, C]

    nsched = np.array([(lens_s > t).sum() for t in range(T)], dtype=np.int64)
    t_eff = int((nsched > 0).sum())
    nsched = nsched[:t_eff]
    nwin = (t_eff + R - 1) // R

    # padded params
    WihP = np.zeros((CPAD, 3 * H), np.float32)
    WihP[:C] = np.asarray(Wih)
    WhhP = np.zeros((CPAD, 3 * H, H), np.float32)
    WhhP[:C] = np.asarray(Whh)
    bihP = np.zeros((CPAD, 3 * H), np.float32)
    bihP[:C] = np.asarray(b_ih)
    bhhP = np.zeros((CPAD, 3 * H), np.float32)
    bhhP[:C] = np.asarray(b_hh)
    WpP = np.zeros((H, CPAD * H), np.float32)
    WpP[:, : C * H] = np.asarray(Wp)

    scale_col = np.concatenate(
        [-np.ones((64, 1), np.float32), np.ones((64, 1), np.float32)]
    )
    ident = np.eye(64, dtype=bfnp)

    in_maps = []
    for k in range(NCORES):
        gs = list(range(k * CH_LOC, (k + 1) * CH_LOC))
        xrow = np.zeros((nwin * R, CH_LOC, B), np.float32)
        wrz = np.zeros((66, CH_LOC * 128), np.float32)
        wnhx = np.zeros((66, CH_LOC * 128), np.float32)
        wp_t = np.zeros((64, CH_LOC * 64), np.float32)
        for c, g in enumerate(gs):
            if g < C:
                xrow[:t_eff, c, :] = x_s[:, :t_eff, g].T
            o = c * 128
            # z block (cols 0:64), r block (cols 64:128)
            wrz[0:64, o : o + 64] = WhhP[g, H : 2 * H, :].T
            wrz[64, o : o + 64] = WihP[g, H : 2 * H]
            wrz[65, o : o + 64] = bihP[g, H : 2 * H] + bhhP[g, H : 2 * H]
            wrz[0:64, o + 64 : o + 128] = WhhP[g, 0:H, :].T
            wrz[64, o + 64 : o + 128] = WihP[g, 0:H]
            wrz[65, o + 64 : o + 128] = bihP[g, 0:H] + bhhP[g, 0:H]
            # nx block (cols 0:64), nh block (cols 64:128)
            wnhx[64, o : o + 64] = WihP[g, 2 * H : 3 * H]
            wnhx[65, o : o + 64] = bihP[g, 2 * H : 3 * H]
            wnhx[0:64, o + 64 : o + 128] = WhhP[g, 2 * H : 3 * H, :].T
            wnhx[65, o + 64 : o + 128] = bhhP[g, 2 * H : 3 * H]
            wp_t[:, c * 64 : (c + 1) * 64] = WpP[:, g * H : (g + 1) * H].T
        # window-packed x: [nwin, 2, R*CH_LOC*B] rows (x, ones)
        xw = np.empty((nwin, 2, W_COLS), np.float32)
        xw[:, 0, :] = xrow.reshape(nwin, R * CH_LOC * B)
        xw[:, 1, :] = 1.0
        in_maps.append(
            {
                "xw": xw.reshape(nwin * 2, W_COLS).astype(bfnp),
                "wrz": wrz.astype(bfnp),
                "wnhx": wnhx.astype(bfnp),
                "wp": wp_t.astype(bfnp),
                "wpred": np.asarray(Wpred, np.float32).reshape(1, 64).T.astype(bfnp),
                "ident": ident,
                "scalecol": scale_col,
                "gammacol": np.asarray(gamma, np.float32).reshape(64, 1),
                "betacol": np.asarray(beta, np.float32).reshape(64, 1),
                "bpredcol": np.asarray(bpred, np.float32).reshape(1, 1),
            }
        )
    return in_maps, perm, t_eff, nsched


_CACHE = {}


def run(inputs, trace=False):
    in_maps, perm, t_eff, nsched = prepare_inputs(**inputs)
    key = (t_eff, tuple(int(v) for v in nsched))
    if key not in _CACHE:
        _CACHE[key] = build_program(t_eff, nsched)
    nc = _CACHE[key]
    res = run_bass_kernel_spmd(
        nc, in_maps, list(range(NCORES)), trace=trace
    )
    out_sorted = np.asarray(res.results[0]["out"][0], np.float32)
    out = np.empty(B, np.float32)
    out[perm] = out_sorted
    return out, res


def kernel(**inputs):
    out, _ = run(inputs, trace=False)
    return out


# revision 22
# speedup vs baseline: 1.4545x; 1.3347x over previous
"""Trainium2 Bass kernel: 34-channel per-channel GRU (input_size=1) over ragged
sequences + concat features -> linear proj -> BatchNorm(train fwd) -> ReLU ->
linear -> sigmoid.

Strategy (v2):
  - Channel-parallel across the 8 NeuronCores: C=34 padded to 40, 5 channels
    per core, full batch B=256 everywhere. Params replicated per-slice.
  - Batch sorted by lens (descending) on the host; at step t only the active
    prefix n_t = #{lens > t} of columns is computed (exact ragged freeze).
  - x rows are staged through two SBUF ring tiles of R=16 step-slots each
    (h in partitions 0:64, x_t in partition 64, ones in partition 65); one DMA
    per 16 steps refills a ring two windows ahead, so no DMA is ever on the
    per-step critical path.
  - The hidden state h lives in the same ring tiles and advances slot-to-slot;
    when columns drop out (sample length reached) the final h for those
    columns is written to a fixed `hfinal` tile instead of the next slot.
  - Channels are split into 3 groups (2,2,1) per core; each group is an
    independent software pipeline, which shortens the per-step dependency
    chain (mm -> sigmoid -> r*nh -> id-matmul accumulate -> tanh -> d,p,h).
  - Per channel, one [66,128] matmul produces [z|r] gate preacts and one
    produces [nx|nh]; contraction rows are [h(64); x_t(1); ones(1)] so the
    input contribution and biases ride in the same matmul.
  - Final features -> partial projection per core -> AllReduce -> BatchNorm
    (batch stats; proj bias cancels in BN) -> ReLU -> pred matvec -> sigmoid.
"""

import sys

sys.path.insert(0, "/opt/trn_rl_repo")

import numpy as np
import ml_dtypes

import concourse.bass as bass
from concourse import bacc, mybir
from concourse.tile import TileContext
from concourse.bass_utils import run_bass_kernel_spmd

B, T, C, H = 256, 512, 34, 64
EPS = 1e-5
NCORES = 8
CPAD = 40  # padded channels
CH_LOC = CPAD // NCORES  # 5
GROUPS = [(0, 2), (2, 2), (4, 1)]  # (start_channel, n_channels) per pipeline
R = 16  # ring slots (steps) per window

BF16 = mybir.dt.bfloat16
F32 = mybir.dt.float32
FP8 = mybir.dt.float8e4
bfnp = ml_dtypes.bfloat16
f8np = ml_dtypes.float8_e4m3
AF = mybir.ActivationFunctionType
OP = mybir.AluOpType

W_COLS = R * CH_LOC * B  # ring tile columns


def _v3(ap2, nch, n, p0=None, p1=None, c0=0):
    """[P, CH*B] AP -> [p0:p1, c0:c0+nch, 0:n] 3D view with B-col channels."""
    a = ap2.rearrange("p (c b) -> p c b", b=B)
    if p0 is None:
        return a[:, c0 : c0 + nch, 0:n]
    return a[p0:p1, c0 : c0 + nch, 0:n]


def _vc3(ap2, nch, n, p0=None, p1=None, lo=0, hi=None):
    """compact work tile [P, W] with channel stride n -> [p, nch, lo:hi]."""
    hi = n if hi is None else hi
    a = ap2[:, 0 : nch * n].rearrange("p (c b) -> p c b", c=nch)
    if p0 is None:
        return a[:, :, lo:hi]
    return a[p0:p1, :, lo:hi]


def build_program(t_eff, nsched):
    nwin = (t_eff + R - 1) // R
    nc = bacc.Bacc(
        "TRN2", target_bir_lowering=False, debug=False, num_devices=NCORES
    )
    xw_d = nc.dram_tensor("xw", [nwin * 2, W_COLS], BF16, kind="ExternalInput").ap()
    wrz_d = nc.dram_tensor("wrz", [66, CH_LOC * 128], FP8, kind="ExternalInput").ap()
    wnhx_d = nc.dram_tensor("wnhx", [66, CH_LOC * 128], FP8, kind="ExternalInput").ap()
    wp_d = nc.dram_tensor("wp", [64, CH_LOC * 64], BF16, kind="ExternalInput").ap()
    wpred_d = nc.dram_tensor("wpred", [64, 1], BF16, kind="ExternalInput").ap()
    ident_d = nc.dram_tensor("ident", [64, 64], FP8, kind="ExternalInput").ap()
    scale_d = nc.dram_tensor("scalecol", [128, 1], F32, kind="ExternalInput").ap()
    gamma_d = nc.dram_tensor("gammacol", [64, 1], F32, kind="ExternalInput").ap()
    beta_d = nc.dram_tensor("betacol", [64, 1], F32, kind="ExternalInput").ap()
    bpred_d = nc.dram_tensor("bpredcol", [1, 1], F32, kind="ExternalInput").ap()
    out_d = nc.dram_tensor("out", [1, B], F32, kind="ExternalOutput").ap()
    cc_in = nc.dram_tensor("cc_in", [64, B], F32).ap()
    cc_out = nc.dram_tensor("cc_out", [64, B], F32, addr_space="Shared").ap()

    with TileContext(nc) as tc:
        with (
            tc.tile_pool(name="const", bufs=1) as cp,
            tc.tile_pool(name="work", bufs=2) as wk,
            tc.tile_pool(name="psum", bufs=1, space="PSUM") as pp,
        ):
            wrz = cp.tile([66, CH_LOC * 128], FP8)
            nc.gpsimd.dma_start(wrz[:], wrz_d[:])
            wnhx = cp.tile([66, CH_LOC * 128], FP8)
            nc.gpsimd.dma_start(wnhx[:], wnhx_d[:])
            ident = cp.tile([64, 64], FP8)
            nc.gpsimd.dma_start(ident[:], ident_d[:])
            scol = cp.tile([128, 1], F32)
            nc.gpsimd.dma_start(scol[:], scale_d[:])

            ring_a = cp.tile([66, W_COLS], BF16)
            ring_b = cp.tile([66, W_COLS], BF16)
            rings = [ring_a, ring_b]
            hfinal = cp.tile([64, CH_LOC * B], BF16)

            # h0 = 0 in ring0 slot 0; x/ones for windows 0,1 via DMA
            nc.vector.memset(rings[0][0:64, 0 : CH_LOC * B], 0.0)
            nc.gpsimd.dma_start(rings[0][64:66, :], xw_d[0:2, :])
            if nwin > 1:
                nc.gpsimd.dma_start(rings[1][64:66, :], xw_d[2:4, :])

            def group_step(t, c0, ng, tiles):
                """Emit one GRU step for channels [c0, c0+ng)."""
                n = int(nsched[t])
                s = t % R
                w = t // R
                ring = rings[w % 2]
                s2 = (t + 1) % R
                ring2 = rings[((t + 1) // R) % 2]
                cb0 = (s * CH_LOC + c0) * B
                nb2 = (s2 * CH_LOC + c0) * B
                hcur = ring[:, cb0 : cb0 + ng * B]
                hnxt = ring2[:, nb2 : nb2 + ng * B]
                gi = c0
                rz0, t10, nh0, d0, p0 = tiles
                rz = rz0[:, c0 * B : (c0 + ng) * B]
                t1 = t10[:, c0 * B : (c0 + ng) * B]
                nh = nh0[:, c0 * B : (c0 + ng) * B]
                d = d0[:, c0 * B : (c0 + ng) * B]
                p = p0[:, c0 * B : (c0 + ng) * B]
                # psum padded to full banks (512 f32 cols) per group;
                # an ng=1 group packs arz|anhx into a single bank (the arz
                # matmul's start=True clears the bank, the nhx matmul then
                # initializes its half via the cleared has_written bits)
                if ng == 1:
                    ab = pp.tile([128, 2 * B], F32, tag=f"ab{gi}")
                    arz = ab[:, 0:B]
                    anhx = ab[:, B : 2 * B]
                else:
                    nbk = 2 * ((ng + 1) // 2)
                    arz = pp.tile([128, nbk * B], F32, tag=f"arz{gi}")
                    anhx = pp.tile([128, nbk * B], F32, tag=f"anhx{gi}")
                for j in range(ng):
                    c = c0 + j
                    nc.tensor.matmul(
                        arz[:, j * B : j * B + n],
                        wrz[:, c * 128 : (c + 1) * 128],
                        ring[0:66, cb0 + j * B : cb0 + j * B + n],
                        start=True,
                        stop=True,
                    )
                for j in range(ng):
                    c = c0 + j
                    # start=True clears has_written for the WHOLE psum bank;
                    # only the first channel of each bank may clear it.
                    nc.tensor.matmul(
                        anhx[:, j * B : j * B + n],
                        wnhx[:, c * 128 : (c + 1) * 128],
                        ring[0:66, cb0 + j * B : cb0 + j * B + n],
                        start=(j % 2 == 0) and ng > 1,
                        stop=False,
                        skip_group_check=True,
                    )
                # sigmoid with per-partition scale [-1]*64 ++ [+1]*64:
                # rows 0:64 = zbar = sig(-Az), rows 64:128 = r = sig(Ar)
                nc.scalar.activation(
                    _v3(rz, ng, n),
                    _v3(arz, ng, n),
                    AF.Sigmoid,
                    scale=scol[:, 0:1],
                )
                # t1 = r * nh   (nh lives on psum partitions 64:128)
                nc.vector.tensor_mul(
                    _v3(t1, ng, n, 0, 64),
                    _v3(rz, ng, n, 64, 128),
                    _v3(anhx, ng, n, 64, 128),
                )
                # nx += I @ t1: one identity-matmul per psum bank pair
                # (psum accumulate, B-strided out; must not cross banks)
                for j0 in range(0, ng, 2):
                    pn = min(2, ng - j0)
                    nc.tensor.matmul(
                        _v3(anhx, pn, n, 0, 64, c0=j0),
                        ident[0:64, :],
                        _v3(t1, pn, n, 0, 64, c0=j0),
                        start=False,
                        stop=True,
                        skip_group_check=True,
                    )
                nc.scalar.activation(
                    _v3(nh, ng, n), _v3(anhx, ng, n, 0, 64), AF.Tanh
                )
                nc.vector.tensor_sub(
                    _v3(d, ng, n), _v3(nh, ng, n), _v3(hcur, ng, n, 0, 64)
                )
                nc.vector.tensor_mul(
                    _v3(p, ng, n), _v3(rz, ng, n, 0, 64), _v3(d, ng, n)
                )
                nc.vector.tensor_add(
                    _v3(hnxt, ng, n, 0, 64),
                    _v3(hcur, ng, n, 0, 64),
                    _v3(p, ng, n),
                )

            # groups run phase-skewed (g1 at t, g2 at t-1, g3 at t-2) so a
            # slow group never head-of-line-blocks another group's ready
            # instructions in the in-order engine queues.
            for t in range(t_eff):
                n = int(nsched[t])
                n2 = int(nsched[t + 1]) if t + 1 < t_eff else 0
                rz0 = wk.tile([128, CH_LOC * B], BF16, tag="rz")
                t10 = wk.tile([64, CH_LOC * B], BF16, tag="t1")
                nh0 = wk.tile([64, CH_LOC * B], BF16, tag="nh")
                d0 = wk.tile([64, CH_LOC * B], BF16, tag="d")
                p0 = wk.tile([64, CH_LOC * B], BF16, tag="p")
                tiles = (rz0, t10, nh0, d0, p0)
                for (c0, ng) in GROUPS:
                    group_step(t, c0, ng, tiles)
                if n2 < n:
                    # retire frozen columns n2:n of all channels to hfinal
                    s2 = (t + 1) % R
                    ring2 = rings[((t + 1) // R) % 2]
                    a_hf = hfinal.rearrange("p (c b) -> p c b", c=CH_LOC)[
                        :, :, n2:n
                    ]
                    a_h2 = ring2[
                        :, s2 * CH_LOC * B : (s2 + 1) * CH_LOC * B
                    ].rearrange("p (c b) -> p c b", c=CH_LOC)[0:64, :, n2:n]
                    nc.gpsimd.tensor_copy(a_hf, a_h2)
                # prefetch x window w+2 into the ring this window just freed
                if t % R == R - 1:
                    w = t // R
                    if w + 2 < nwin:
                        nc.gpsimd.dma_start(
                            rings[w % 2][64:66, :],
                            xw_d[2 * (w + 2) : 2 * (w + 2) + 2, :],
                        )

            # ---- tail: proj partial -> allreduce -> BN -> relu -> pred ----
            wp = cp.tile([64, CH_LOC * 64], BF16)
            nc.gpsimd.dma_start(wp[:], wp_d[:])
            pj = pp.tile([64, B], F32, tag="pproj")
            for c in range(CH_LOC):
                nc.tensor.matmul(
                    pj[:, :],
                    wp[:, c * 64 : (c + 1) * 64],
                    hfinal[0:64, c * B : (c + 1) * B],
                    start=(c == 0),
                    stop=(c == CH_LOC - 1),
                )
            pjs = cp.tile([64, B], F32)
            nc.scalar.copy(pjs[:], pj[:])
            nc.gpsimd.dma_start(cc_in[:], pjs[:])
            nc.gpsimd.collective_compute(
                "AllReduce",
                OP.add,
                replica_groups=[list(range(NCORES))],
                ins=[cc_in[:]],
                outs=[cc_out[:]],
            )
            prj = cp.tile([64, B], F32)
            nc.gpsimd.dma_start(prj[:], cc_out[:])

            musum = cp.tile([64, 1], F32)
            nc.vector.tensor_reduce(musum[:], prj[:], mybir.AxisListType.X, OP.add)
            mu = cp.tile([64, 1], F32)
            nc.scalar.mul(mu[:], musum[:], 1.0 / B)
            cen = cp.tile([64, B], F32)
            nc.vector.tensor_scalar_sub(cen[:], prj[:], mu[:, 0:1])
            sq = cp.tile([64, B], F32)
            nc.vector.tensor_mul(sq[:], cen[:], cen[:])
            vsum = cp.tile([64, 1], F32)
            nc.vector.tensor_reduce(vsum[:], sq[:], mybir.AxisListType.X, OP.add)
            v = cp.tile([64, 1], F32)
            nc.scalar.mul(v[:], vsum[:], 1.0 / B)
            veps = cp.tile([64, 1], F32)
            nc.vector.tensor_scalar_add(veps[:], v[:], EPS)
            std = cp.tile([64, 1], F32)
            nc.scalar.activation(std[:], veps[:], AF.Sqrt)
            rstd = cp.tile([64, 1], F32)
            nc.vector.reciprocal(rstd[:], std[:])
            gam = cp.tile([64, 1], F32)
            nc.gpsimd.dma_start(gam[:], gamma_d[:])
            bet = cp.tile([64, 1], F32)
            nc.gpsimd.dma_start(bet[:], beta_d[:])
            sc2 = cp.tile([64, 1], F32)
            nc.vector.tensor_mul(sc2[:], rstd[:], gam[:])
            y = cp.tile([64, B], BF16)
            nc.vector.tensor_scalar(
                y[:], cen[:], sc2[:, 0:1], bet[:, 0:1], OP.mult, OP.add
            )
            yr = cp.tile([64, B], BF16)
            nc.vector.tensor_scalar_max(yr[:], y[:], 0.0)
            wpred = cp.tile([64, 1], BF16)
            nc.gpsimd.dma_start(wpred[:], wpred_d[:])
            pps = pp.tile([1, B], F32, tag="pred")
            nc.tensor.matmul(pps[:], wpred[:, 0:1], yr[:, :], start=True, stop=True)
            bp = cp.tile([1, 1], F32)
            nc.gpsimd.dma_start(bp[:], bpred_d[:])
            osb = cp.tile([1, B], F32)
            nc.scalar.activation(osb[:], pps[:], AF.Sigmoid, bias=bp[0:1, 0:1])
            nc.gpsimd.dma_start(out_d[:], osb[:])

    nc.compile()
    return nc


def prepare_inputs(x, Wih, Whh, b_ih, b_hh, Wp, bp, gamma, beta, Wpred, bpred, lens):
    """Host-side: sort batch by lens desc, pack per-core tensors."""
    x = np.asarray(x)
    lens = np.asarray(lens)
    perm = np.argsort(-lens, kind="stable")
    lens_s = lens[perm]
    x_s = x[perm]  # [B, T, C]

    nsched = np.array([(lens_s > t).sum() for t in range(T)], dtype=np.int64)
    t_eff = int((nsched > 0).sum())
    nsched = nsched[:t_eff]
    nwin = (t_eff + R - 1) // R

    # padded params
    WihP = np.zeros((CPAD, 3 * H), np.float32)
    WihP[:C] = np.asarray(Wih)
    WhhP = np.zeros((CPAD, 3 * H, H), np.float32)
    WhhP[:C] = np.asarray(Whh)
    bihP = np.zeros((CPAD, 3 * H), np.float32)
    bihP[:C] = np.asarray(b_ih)
    bhhP = np.zeros((CPAD, 3 * H), np.float32)
    bhhP[:C] = np.asarray(b_hh)
    WpP = np.zeros((H, CPAD * H), np.float32)
    WpP[:, : C * H] = np.asarray(Wp)

    scale_col = np.concatenate(
        [-np.ones((64, 1), np.float32), np.ones((64, 1), np.float32)]
    )
    ident = np.eye(64, dtype=f8np)

    in_maps = []
    for k in range(NCORES):
        gs = list(range(k * CH_LOC, (k + 1) * CH_LOC))
        xrow = np.zeros((nwin * R, CH_LOC, B), np.float32)
        wrz = np.zeros((66, CH_LOC * 128), np.float32)
        wnhx = np.zeros((66, CH_LOC * 128), np.float32)
        wp_t = np.zeros((64, CH_LOC * 64), np.float32)
        for c, g in enumerate(gs):
            if g < C:
                xrow[:t_eff, c, :] = x_s[:, :t_eff, g].T
            o = c * 128
            # z block (cols 0:64), r block (cols 64:128)
            wrz[0:64, o : o + 64] = WhhP[g, H : 2 * H, :].T
            wrz[64, o : o + 64] = WihP[g, H : 2 * H]
            wrz[65, o : o + 64] = bihP[g, H : 2 * H] + bhhP[g, H : 2 * H]
            wrz[0:64, o + 64 : o + 128] = WhhP[g, 0:H, :].T
            wrz[64, o + 64 : o + 128] = WihP[g, 0:H]
            wrz[65, o + 64 : o + 128] = bihP[g, 0:H] + bhhP[g, 0:H]
            # nx block (cols 0:64), nh block (cols 64:128)
            wnhx[64, o : o + 64] = WihP[g, 2 * H : 3 * H]
            wnhx[65, o : o + 64] = bihP[g, 2 * H : 3 * H]
            wnhx[0:64, o + 64 : o + 128] = WhhP[g, 2 * H : 3 * H, :].T
            wnhx[65, o + 64 : o + 128] = bhhP[g, 2 * H : 3 * H]
            wp_t[:, c * 64 : (c + 1) * 64] = WpP[:, g * H : (g + 1) * H].T
        # window-packed x: [nwin, 2, R*CH_LOC*B] rows (x, ones)
        xw = np.empty((nwin, 2, W_COLS), np.float32)
        xw[:, 0, :] = xrow.reshape(nwin, R * CH_LOC * B)
        xw[:, 1, :] = 1.0
        in_maps.append(
            {
                "xw": xw.reshape(nwin * 2, W_COLS).astype(bfnp),
                "wrz": wrz.astype(f8np),
                "wnhx": wnhx.astype(f8np),
                "wp": wp_t.astype(bfnp),
                "wpred": np.asarray(Wpred, np.float32).reshape(1, 64).T.astype(bfnp),
                "ident": ident,
                "scalecol": scale_col,
                "gammacol": np.asarray(gamma, np.float32).reshape(64, 1),
                "betacol": np.asarray(beta, np.float32).reshape(64, 1),
                "bpredcol": np.asarray(bpred, np.float32).reshape(1, 1),
            }
        )
    return in_maps, perm, t_eff, nsched


_CACHE = {}


def run(inputs, trace=False):
    in_maps, perm, t_eff, nsched = prepare_inputs(**inputs)
    key = (t_eff, tuple(int(v) for v in nsched))
    if key not in _CACHE:
        _CACHE[key] = build_program(t_eff, nsched)
    nc = _CACHE[key]
    res = run_bass_kernel_spmd(
        nc, in_maps, list(range(NCORES)), trace=trace
    )
    out_sorted = np.asarray(res.results[0]["out"][0], np.float32)
    out = np.empty(B, np.float32)
    out[perm] = out_sorted
    return out, res


def kernel(**inputs):
    out, _ = run(inputs, trace=False)
    return out
